# revision 6
# baseline (speedup 1.0000x reference)
"""TRN2 Bass kernel for nn_Attention_5720896438407 (8-core data-parallel).

Mathematical collapse: the module computes SDPA over the *head* axis with a
single KV head (KV=1), so the softmax runs over a size-1 axis and every
attention weight is exactly 1.0.  The q path (q_a/q_norm/q_b), both rotary
embeddings, the nope/rope blend and the attention mask all cancel out, and
the module reduces to

    T  = hidden @ kv_a_w.T + kv_a_b                    # (ntok, 512)
    s  = rsqrt(mean(T^2, -1) + eps)                    # per-token RMS scale
    V  = (s*T) @ (kv_b_w[128:] * (1 + kv_norm_w)).T + kv_b_b[128:]
    Y  = V @ M.T      with  M = o_w.reshape(2048, 16, 128).sum(1)

(the attention output tiles V across all 16 heads, so o_proj sees the head
sum of its weight).

Distribution: pure data-parallel over the 8192 tokens — 1024 tokens per
NeuronCore, no collectives.  Per core the tokens stream through in 8 slabs
of 128 tokens.

Schedule notes (tuned from NTFF profiles):
- The per-token RMS scale commutes through both remaining matmuls (it is a
  row scaling), so when kv_b_b == 0 it is applied at the very end, in the
  PSUM->SBUF copies of Y where tokens sit on partitions.  This takes the
  serial Square->Sqrt->reciprocal chain off the tail critical path: the PE
  transposes depend only on a plain fp16 copy of T.
- The PE HAM clock gate needs ~3.4us of sustained activity to reach 2.4GHz.
  Slab 0's step-1 matmuls trickle at input-DMA pace (the w1 stream is the
  prologue bottleneck), so junk matmuls on a zero tile are interleaved
  after each slab-0 chunk to keep the duty cycle high; by the time the
  input stream catches up the clock is warm and never re-throttles.
- Input on the SP HWDGE ring: w1 quarters interleaved with slab-0 quarters,
  then slab 1, then slabs 2..7 as three 1MB pair transfers.  Small weights
  and output DMAs ride the ACT ring.
Step-1 operands are fp16 (fp32 PSUM accumulation), RMS statistics in fp32,
downstream matmuls fp16, Y ships as fp16 (host casts back to fp32).
End-to-end error vs the fp32 reference is ~5e-4 relative.
"""
import sys

sys.path.insert(0, "/opt/trn_rl_repo")

import numpy as np
import concourse.bass as bass
import concourse.tile as tile
from concourse import bacc, mybir
from concourse.bass_utils import run_bass_kernel_spmd
from concourse.masks import make_identity

F32 = mybir.dt.float32
F16 = mybir.dt.float16

HID = 2048
KV = 512
D = 128
OUT = 2048
EPS = 1e-6
N_HID_CK = HID // 128   # 16
N_KV_CK = KV // 128     # 4
N_OUT_T = OUT // 512    # 4
SLAB = 128              # tokens per slab
N_CORES = 8
AF = mybir.ActivationFunctionType

SLAB0_JUNK = 6          # junk matmuls after each slab-0 chunk (HAM warm-up)

_NC_CACHE = {}


def _build_nc(tok, with_ba, with_bv):
    nslab = tok // SLAB
    assert tok % SLAB == 0 and nslab % 2 == 0 and nslab >= 4

    nc = bacc.Bacc("TRN2", target_bir_lowering=False, debug=False,
                   num_devices=1)

    # token slabs, pair-major: [pair, hid_row, sub_slab, hid_chunk, token]
    xts_d = nc.dram_tensor("xts", (nslab // 2, 128, 2, N_HID_CK, SLAB), F16,
                           kind="ExternalInput").ap()
    w1s_d = nc.dram_tensor("w1s", (128, N_HID_CK, KV), F16,
                           kind="ExternalInput").ap()
    wvt_d = nc.dram_tensor("wvt", (KV, D), F16, kind="ExternalInput").ap()
    mt_d = nc.dram_tensor("mt", (D, OUT), F16, kind="ExternalInput").ap()
    bv_d = nc.dram_tensor("bv", (D, 1), F32, kind="ExternalInput").ap()
    if with_ba:
        ba_d = nc.dram_tensor("bar", (1, KV), F16, kind="ExternalInput").ap()
        onesr_d = nc.dram_tensor("onesr", (1, 128), F16,
                                 kind="ExternalInput").ap()
    y_d = nc.dram_tensor("y", (tok, OUT), F16, kind="ExternalOutput").ap()

    with tile.TileContext(nc) as tc:
        with tc.tile_pool(name="consts", bufs=1) as consts, \
             tc.tile_pool(name="s01", bufs=2) as s01, \
             tc.tile_pool(name="pairs", bufs=max(1, nslab // 2 - 1)) as pairs, \
             tc.tile_pool(name="work", bufs=2) as work, \
             tc.tile_pool(name="ps_t", bufs=3, space="PSUM") as ps_t, \
             tc.tile_pool(name="ps_r", bufs=2, space="PSUM") as ps_r, \
             tc.tile_pool(name="ps_v", bufs=1, space="PSUM") as ps_v, \
             tc.tile_pool(name="ps_y", bufs=2, space="PSUM") as ps_y:
            # zero tile + PSUM scratch for HAM warm-up junk matmuls
            js = consts.tile([128, 128], F16, tag="js")
            nc.vector.memset(js[:], 0.0)
            junka = ps_y.tile([128, 512], F32, tag="py", name="junka")
            junkb = ps_y.tile([128, 512], F32, tag="py", name="junkb")
            jstate = [0]

            def junk_mm(n=1):
                for _ in range(n):
                    t = junka if jstate[0] % 2 == 0 else junkb
                    jstate[0] += 1
                    nc.tensor.matmul(t[:, 0:128], js[:], js[:],
                                     start=True, stop=True)

            # ---- input stream on the SP ring, in priority order:
            #      W1 quarters interleaved with slab-0 quarters, then slab 1,
            #      then slab pairs (1MB each) ----
            w1_s = consts.tile([128, N_HID_CK, KV], F16, tag="w1")
            s0 = s01.tile([128, N_HID_CK, SLAB], F16, tag="s01", name="s0")
            s1 = s01.tile([128, N_HID_CK, SLAB], F16, tag="s01", name="s1")
            for h in range(4):
                ck = slice(4 * h, 4 * h + 4)
                nc.sync.dma_start(w1_s[:, ck, :], w1s_d[:, ck, :])
                nc.sync.dma_start(s0[:, ck, :], xts_d[0, :, 0, ck, :])
            nc.sync.dma_start(s1[:], xts_d[0, :, 1])
            pair_tiles = []
            for p in range(1, nslab // 2):
                t = pairs.tile([128, 2, N_HID_CK, SLAB], F16, tag="pair",
                               name=f"pair{p}")
                nc.sync.dma_start(t[:], xts_d[p])
                pair_tiles.append(t)

            def sg(g):
                if g == 0:
                    return s0
                if g == 1:
                    return s1
                p, j = divmod(g, 2)
                return pair_tiles[p - 1][:, j]

            # ---- small constants + all output DMAs on the ACT ring ----
            wv_s = []
            for c in range(N_KV_CK):
                t = consts.tile([128, D], F16, tag=f"wv_{c}", name=f"wv_{c}")
                nc.scalar.dma_start(t[:], wvt_d[c * 128:(c + 1) * 128, :])
                wv_s.append(t)
            mt_s = consts.tile([128, OUT], F16, tag="mt")
            nc.scalar.dma_start(mt_s[:], mt_d)
            bv_s = consts.tile([128, 1], F32, tag="bv")
            nc.scalar.dma_start(bv_s[:], bv_d)
            if with_ba:
                ba_s = consts.tile([1, KV], F16, tag="ba")
                nc.scalar.dma_start(ba_s[:], ba_d)
                onesr_s = consts.tile([1, 128], F16, tag="onesr")
                nc.scalar.dma_start(onesr_s[:], onesr_d)

            # identity for the PE transposes (ready long before tail(0))
            ident = consts.tile([128, 128], F16, tag="ident")
            make_identity(nc, ident[:])
            eps_s = consts.tile([128, 1], F32, tag="eps")
            nc.vector.memset(eps_s[:], EPS)

            def step1(g, junk_per_chunk=0):
                # T.T slab accumulation, token-major: 16 chunk matmuls,
                # fp16 operands, fp32 PSUM.  During slab 0's DMA-trickle
                # phase, junk matmuls after each chunk keep the PE duty
                # cycle high enough that the HAM clock gate opens and
                # stays open.
                pt = ps_t.tile([128, KV], F32, tag="pt", name=f"pt{g}")
                for ck in range(N_HID_CK):
                    nc.tensor.matmul(
                        pt[:], sg(g)[:, ck, :], w1_s[:, ck, :],
                        start=(ck == 0),
                        stop=(ck == N_HID_CK - 1 and not with_ba),
                    )
                    junk_mm(junk_per_chunk)
                if with_ba:
                    # rank-1 row-broadcast of kv_a_b into the accumulation
                    nc.tensor.matmul(pt[:], onesr_s[:], ba_s[:],
                                     start=False, stop=True)
                return pt

            def stat(g, pt):
                # RMS statistics + fp16 staging copy of T.  On the
                # deferred-scale path nothing downstream waits on the
                # Square/Sqrt/reciprocal chain except the final Y copies.
                sqj = work.tile([128, KV], F32, tag="sqj")
                ssq = work.tile([128, 1], F32, tag="ssq")
                nc.scalar.activation(sqj[:], pt[:], AF.Square,
                                     accum_out=ssq[:])
                rt = work.tile([128, 1], F32, tag="rt")
                nc.scalar.activation(rt[:], ssq[:], AF.Sqrt,
                                     bias=eps_s[:], scale=1.0 / KV)
                sc = work.tile([128, 1], F32, tag="sc", bufs=4)
                nc.vector.reciprocal(sc[:], rt[:])
                ttn = work.tile([128, KV], F16, tag="ttn", bufs=3)
                if with_bv:
                    # kv_b_b != 0: scale must be applied before the V bias,
                    # so multiply T by s before the transposes (old path).
                    nc.vector.tensor_scalar_mul(ttn[:], pt[:], sc[:])
                else:
                    # deferred-scale path: plain fp16 copy of T; the RMS
                    # scale rides the final Y copies (per-partition there)
                    nc.vector.tensor_copy(ttn[:], pt[:])
                return ttn, sc

            def tailpe(g, ttn, sc):
                t0 = g * SLAB
                last = g == nslab - 1
                # transpose into kv-major for step 2
                trp = ps_r.tile([128, N_KV_CK, SLAB], F16, tag="trp",
                                name=f"trp{g}")
                for c in range(N_KV_CK):
                    nc.tensor.transpose(trp[:, c, :],
                                        ttn[:, c * 128:(c + 1) * 128],
                                        ident[:])
                ttr = work.tile([128, N_KV_CK, SLAB], F16, tag="ttr", bufs=3)
                nc.vector.tensor_copy(ttr[:], trp[:])
                # step 2: V.T = Wv' @ T.T (unscaled on the fast path)
                vtp = ps_v.tile([128, SLAB], F32, tag="vtp", name=f"vtp{g}")
                for c in range(N_KV_CK):
                    nc.tensor.matmul(vtp[:], wv_s[c][:], ttr[:, c, :],
                                     start=(c == 0),
                                     stop=(c == N_KV_CK - 1))
                vts = work.tile([128, SLAB], F16, tag="vts", bufs=3)
                if with_bv:
                    nc.scalar.activation(vts[:], vtp[:], AF.Identity,
                                         bias=bv_s[:], scale=1.0)
                else:
                    nc.scalar.activation(vts[:], vtp[:], AF.Copy,
                                         bias=0.0, scale=1.0)
                # step 4: Y = V @ M.T; on the fast path the RMS scale is
                # applied here (tokens are partitions in py/ysb).
                ysb = work.tile([128, OUT], F16, tag="ysb", bufs=6)
                for n in range(N_OUT_T):
                    py = ps_y.tile([128, 512], F32, tag="py",
                                   name=f"py{g}_{n}")
                    nc.tensor.matmul(py[:], vts[:],
                                     mt_s[:, n * 512:(n + 1) * 512],
                                     start=True, stop=True)
                    ysl = ysb[:, n * 512:(n + 1) * 512]
                    if with_bv:
                        if n % 2 == 0:
                            nc.vector.tensor_copy(ysl, py[:])
                        else:
                            nc.scalar.activation(ysl, py[:], AF.Copy,
                                                 bias=0.0, scale=1.0)
                    else:
                        if n % 2 == 0:
                            nc.vector.tensor_scalar_mul(ysl, py[:], sc[:])
                        else:
                            nc.scalar.activation(ysl, py[:], AF.Copy,
                                                 bias=0.0, scale=sc[:])
                    if last and n == 1:
                        # final slab: overlap the first output half with the
                        # remaining matmuls/copies so only 0.25 MB trails
                        nc.scalar.dma_start(y_d[t0:t0 + SLAB, 0:1024],
                                            ysb[:, 0:1024])
                if last:
                    nc.scalar.dma_start(y_d[t0:t0 + SLAB, 1024:2048],
                                        ysb[:, 1024:2048])
                else:
                    nc.scalar.dma_start(y_d[t0:t0 + SLAB, :], ysb[:])

            # 3-stage software pipeline: step1(g) | stat(g-1) | tailpe(g-2).
            # The stats/staging chain of slab g runs during step1(g+1), and
            # slab g's PE tail runs during step1(g+2), so neither the PE nor
            # the drain ever waits on a cross-engine round trip.
            pts = {}
            stats = {}
            for g in range(nslab):
                pts[g] = step1(g, junk_per_chunk=(SLAB0_JUNK if g == 0 else 0))
                if g >= 1:
                    stats[g - 1] = stat(g - 1, pts.pop(g - 1))
                if g >= 2:
                    tailpe(g - 2, *stats.pop(g - 2))
            stats[nslab - 1] = stat(nslab - 1, pts.pop(nslab - 1))
            tailpe(nslab - 2, *stats.pop(nslab - 2))
            tailpe(nslab - 1, *stats.pop(nslab - 1))

    nc.compile()
    return nc


def _host_prep(inputs):
    """Fold weights, swizzle X into fp16 token slabs, shard across cores."""
    h = np.asarray(inputs["hidden_states"], dtype=np.float32)
    b, s, hid = h.shape
    assert hid == HID
    x = np.ascontiguousarray(h.reshape(b * s, hid))
    ntok = b * s
    tok = ntok // N_CORES
    nslab = tok // SLAB

    kv_a_w = np.asarray(inputs["kv_a_w"], np.float32)
    kv_a_b = np.asarray(inputs["kv_a_b"], np.float32)
    kv_norm_w = np.asarray(inputs["kv_norm_w"], np.float32)
    kv_b_w = np.asarray(inputs["kv_b_w"], np.float32)
    kv_b_b = np.asarray(inputs["kv_b_b"], np.float32)
    o_w = np.asarray(inputs["o_w"], np.float32)

    w1s = np.ascontiguousarray(
        kv_a_w.T.reshape(N_HID_CK, 128, KV).transpose(1, 0, 2)
    ).astype(np.float16)
    wv = kv_b_w[D:2 * D] * (1.0 + kv_norm_w)[None, :]
    wvt = np.ascontiguousarray(wv.T).astype(np.float16)
    M = o_w.reshape(HID, 16, D).sum(axis=1)
    mt = np.ascontiguousarray(M.T).astype(np.float16)
    bv = np.ascontiguousarray(kv_b_b[D:2 * D].reshape(D, 1)).astype(np.float32)
    with_ba = bool(np.any(kv_a_b != 0.0))
    with_bv = bool(np.any(kv_b_b[D:2 * D] != 0.0))
    ba_row = np.ascontiguousarray(kv_a_b.reshape(1, KV)).astype(np.float16)
    ones_row = np.ones((1, 128), np.float16)

    in_maps = []
    for i in range(N_CORES):
        shard = x[i * tok:(i + 1) * tok]
        # [pair, hid_row, sub_slab, hid_chunk, token]
        xts = np.ascontiguousarray(
            shard.T.reshape(N_HID_CK, 128, nslab // 2, 2, SLAB)
            .transpose(2, 1, 3, 0, 4)
        ).astype(np.float16)
        m = {"xts": xts, "w1s": w1s, "wvt": wvt, "mt": mt, "bv": bv}
        if with_ba:
            m["bar"] = ba_row
            m["onesr"] = ones_row
        in_maps.append(m)

    def gather(results):
        y = np.concatenate([r["y"] for r in results], axis=0)
        return np.ascontiguousarray(y.reshape(b, s, HID).astype(np.float32))

    return in_maps, gather, with_ba, with_bv, tok


def _run(inputs, trace=False, **spmd_kwargs):
    in_maps, gather, with_ba, with_bv, tok = _host_prep(inputs)
    key = (tok, with_ba, with_bv)
    if key not in _NC_CACHE:
        _NC_CACHE[key] = _build_nc(tok, with_ba, with_bv)
    nc = _NC_CACHE[key]
    res = run_bass_kernel_spmd(nc, in_maps, core_ids=list(range(N_CORES)),
                               trace=trace, **spmd_kwargs)
    return gather(res.results), res


def kernel(**inputs) -> np.ndarray:
    y, _ = _run(inputs, trace=False)
    return y


# revision 9
# speedup vs baseline: 1.2063x; 1.2063x over previous
"""TRN2 Bass kernel for nn_Attention_5720896438407 (8-core data-parallel).

Mathematical collapse: the module computes SDPA over the *head* axis with a
single KV head (KV=1), so the softmax runs over a size-1 axis and every
attention weight is exactly 1.0.  The q path (q_a/q_norm/q_b), both rotary
embeddings, the nope/rope blend and the attention mask all cancel out, and
the module reduces to

    T  = hidden @ kv_a_w.T + kv_a_b                    # (ntok, 512)
    s  = rsqrt(mean(T^2, -1) + eps)                    # per-token RMS scale
    V  = (s*T) @ (kv_b_w[128:] * (1 + kv_norm_w)).T + kv_b_b[128:]
    Y  = V @ M.T      with  M = o_w.reshape(2048, 16, 128).sum(1)

(the attention output tiles V across all 16 heads, so o_proj sees the head
sum of its weight).

Distribution: pure data-parallel over the 8192 tokens — 1024 tokens per
NeuronCore, no collectives.  Per core the tokens stream through in 8 slabs
of 128 tokens.

Schedule notes (tuned from NTFF profiles):
- The per-token RMS scale commutes through both remaining matmuls (it is a
  row scaling), so when kv_b_b == 0 it is applied at the very end, in the
  PSUM->SBUF copies of Y where tokens sit on partitions.  This takes the
  serial Square->Sqrt->reciprocal chain off the tail critical path: the PE
  transposes depend only on a plain fp16 copy of T.
- The PE HAM clock gate needs ~3.4us of sustained activity to reach 2.4GHz.
  Slab 0's step-1 matmuls trickle at input-DMA pace (the w1 stream is the
  prologue bottleneck), so junk matmuls on a zero tile are interleaved
  after each slab-0 chunk to keep the duty cycle high; by the time the
  input stream catches up the clock is warm and never re-throttles.
- Input on the SP HWDGE ring: w1 quarters interleaved with slab-0 quarters,
  then slab 1, then slabs 2..7 as three 1MB pair transfers.  Small weights
  and output DMAs ride the ACT ring.
Step-1 operands are fp16 (fp32 PSUM accumulation), RMS statistics in fp32,
downstream matmuls fp16, Y ships as fp16 (host casts back to fp32).
End-to-end error vs the fp32 reference is ~5e-4 relative.
"""
import sys

sys.path.insert(0, "/opt/trn_rl_repo")

import numpy as np
import concourse.bass as bass
import concourse.tile as tile
from concourse import bacc, mybir
from concourse.bass_utils import run_bass_kernel_spmd
from concourse.masks import make_identity

F32 = mybir.dt.float32
F16 = mybir.dt.float16

HID = 2048
KV = 512
D = 128
OUT = 2048
EPS = 1e-6
N_HID_CK = HID // 128   # 16
N_KV_CK = KV // 128     # 4
N_OUT_T = OUT // 512    # 4
SLAB = 128              # tokens per slab
N_CORES = 8
AF = mybir.ActivationFunctionType

SLAB0_JUNK = 6          # junk matmuls after each slab-0 chunk (HAM warm-up)

_NC_CACHE = {}


def _build_nc(tok, with_ba, with_bv):
    nslab = tok // SLAB
    assert tok % SLAB == 0 and nslab % 2 == 0 and nslab >= 4

    nc = bacc.Bacc("TRN2", target_bir_lowering=False, debug=False,
                   num_devices=1)

    # token slabs, pair-major: [pair, hid_row, sub_slab, hid_chunk, token]
    xts_d = nc.dram_tensor("xts", (nslab // 2, 128, 2, N_HID_CK, SLAB), F16,
                           kind="ExternalInput").ap()
    w1s_d = nc.dram_tensor("w1s", (128, N_HID_CK, KV), F16,
                           kind="ExternalInput").ap()
    wvt_d = nc.dram_tensor("wvt", (KV, D), F16, kind="ExternalInput").ap()
    mt_d = nc.dram_tensor("mt", (D, OUT), F16, kind="ExternalInput").ap()
    bv_d = nc.dram_tensor("bv", (D, 1), F32, kind="ExternalInput").ap()
    if with_ba:
        ba_d = nc.dram_tensor("bar", (1, KV), F16, kind="ExternalInput").ap()
        onesr_d = nc.dram_tensor("onesr", (1, 128), F16,
                                 kind="ExternalInput").ap()
    y_d = nc.dram_tensor("y", (tok, OUT), F16, kind="ExternalOutput").ap()

    with tile.TileContext(nc) as tc:
        with tc.tile_pool(name="consts", bufs=1) as consts, \
             tc.tile_pool(name="s01", bufs=2) as s01, \
             tc.tile_pool(name="pairs", bufs=max(1, nslab // 2 - 1)) as pairs, \
             tc.tile_pool(name="work", bufs=2) as work, \
             tc.tile_pool(name="ps_t", bufs=2, space="PSUM") as ps_t, \
             tc.tile_pool(name="ps_r", bufs=1, space="PSUM") as ps_r, \
             tc.tile_pool(name="ps_v", bufs=1, space="PSUM") as ps_v, \
             tc.tile_pool(name="ps_y", bufs=4, space="PSUM") as ps_y:
            # zero tile + PSUM scratch for HAM warm-up junk matmuls
            js = consts.tile([128, 128], F16, tag="js")
            nc.vector.memset(js[:], 0.0)
            junka = ps_y.tile([128, 512], F32, tag="py", name="junka")
            junkb = ps_y.tile([128, 512], F32, tag="py", name="junkb")
            jstate = [0]

            def junk_mm(n=1):
                for _ in range(n):
                    t = junka if jstate[0] % 2 == 0 else junkb
                    jstate[0] += 1
                    nc.tensor.matmul(t[:, 0:128], js[:], js[:],
                                     start=True, stop=True)

            # ---- input stream on the SP ring, in priority order:
            #      W1 quarters interleaved with slab-0 quarters, then slab 1,
            #      then slab pairs (1MB each) ----
            w1_s = consts.tile([128, N_HID_CK, KV], F16, tag="w1")
            s0 = s01.tile([128, N_HID_CK, SLAB], F16, tag="s01", name="s0")
            s1 = s01.tile([128, N_HID_CK, SLAB], F16, tag="s01", name="s1")
            for h in range(4):
                ck = slice(4 * h, 4 * h + 4)
                nc.sync.dma_start(w1_s[:, ck, :], w1s_d[:, ck, :])
                nc.sync.dma_start(s0[:, ck, :], xts_d[0, :, 0, ck, :])
            nc.sync.dma_start(s1[:], xts_d[0, :, 1])
            pair_tiles = []
            for p in range(1, nslab // 2):
                t = pairs.tile([128, 2, N_HID_CK, SLAB], F16, tag="pair",
                               name=f"pair{p}")
                nc.sync.dma_start(t[:], xts_d[p])
                pair_tiles.append(t)

            def sg(g):
                if g == 0:
                    return s0
                if g == 1:
                    return s1
                p, j = divmod(g, 2)
                return pair_tiles[p - 1][:, j]

            # ---- small constants + all output DMAs on the ACT ring ----
            wv_s = []
            for c in range(N_KV_CK):
                t = consts.tile([128, D], F16, tag=f"wv_{c}", name=f"wv_{c}")
                nc.scalar.dma_start(t[:], wvt_d[c * 128:(c + 1) * 128, :])
                wv_s.append(t)
            mt_s = consts.tile([128, OUT], F16, tag="mt")
            nc.scalar.dma_start(mt_s[:], mt_d)
            bv_s = consts.tile([128, 1], F32, tag="bv")
            nc.scalar.dma_start(bv_s[:], bv_d)
            if with_ba:
                ba_s = consts.tile([1, KV], F16, tag="ba")
                nc.scalar.dma_start(ba_s[:], ba_d)
                onesr_s = consts.tile([1, 128], F16, tag="onesr")
                nc.scalar.dma_start(onesr_s[:], onesr_d)

            # identity for the PE transposes (ready long before tail(0))
            ident = consts.tile([128, 128], F16, tag="ident")
            make_identity(nc, ident[:])
            eps_s = consts.tile([128, 1], F32, tag="eps")
            nc.vector.memset(eps_s[:], EPS)

            def step1(g, junk_per_chunk=0):
                # T.T slab accumulation, token-major: 16 chunk matmuls,
                # fp16 operands, fp32 PSUM.  During slab 0's DMA-trickle
                # phase, junk matmuls after each chunk keep the PE duty
                # cycle high enough that the HAM clock gate opens and
                # stays open.
                pt = ps_t.tile([128, KV], F32, tag="pt", name=f"pt{g}")
                for ck in range(N_HID_CK):
                    nc.tensor.matmul(
                        pt[:], sg(g)[:, ck, :], w1_s[:, ck, :],
                        start=(ck == 0),
                        stop=(ck == N_HID_CK - 1 and not with_ba),
                    )
                    junk_mm(junk_per_chunk)
                if with_ba:
                    # rank-1 row-broadcast of kv_a_b into the accumulation
                    nc.tensor.matmul(pt[:], onesr_s[:], ba_s[:],
                                     start=False, stop=True)
                return pt

            def stat(g, pt):
                # RMS statistics + fp16 staging copy of T.  On the
                # deferred-scale path nothing downstream waits on the
                # Square/Sqrt/reciprocal chain except the final Y copies.
                sqj = work.tile([128, KV], F32, tag="sqj")
                ssq = work.tile([128, 1], F32, tag="ssq")
                nc.scalar.activation(sqj[:], pt[:], AF.Square,
                                     accum_out=ssq[:])
                rt = work.tile([128, 1], F32, tag="rt")
                nc.scalar.activation(rt[:], ssq[:], AF.Sqrt,
                                     bias=eps_s[:], scale=1.0 / KV)
                sc = work.tile([128, 1], F32, tag="sc", bufs=4)
                nc.vector.reciprocal(sc[:], rt[:])
                ttn = work.tile([128, KV], F16, tag="ttn", bufs=3)
                if with_bv:
                    # kv_b_b != 0: scale must be applied before the V bias,
                    # so multiply T by s before the transposes (old path).
                    nc.vector.tensor_scalar_mul(ttn[:], pt[:], sc[:])
                else:
                    # deferred-scale path: plain fp16 copy of T; the RMS
                    # scale rides the final Y copies (per-partition there)
                    nc.vector.tensor_copy(ttn[:], pt[:])
                return ttn, sc

            def tail_a(g, ttn):
                # transpose into kv-major, step 2, and the V staging copy
                trp = ps_r.tile([128, N_KV_CK, SLAB], F16, tag="trp",
                                name=f"trp{g}")
                for c in range(N_KV_CK):
                    nc.tensor.transpose(trp[:, c, :],
                                        ttn[:, c * 128:(c + 1) * 128],
                                        ident[:])
                ttr = work.tile([128, N_KV_CK, SLAB], F16, tag="ttr", bufs=3)
                nc.vector.tensor_copy(ttr[:], trp[:])
                # step 2: V.T = Wv' @ T.T (unscaled on the fast path)
                vtp = ps_v.tile([128, SLAB], F32, tag="vtp", name=f"vtp{g}")
                for c in range(N_KV_CK):
                    nc.tensor.matmul(vtp[:], wv_s[c][:], ttr[:, c, :],
                                     start=(c == 0),
                                     stop=(c == N_KV_CK - 1))
                vts = work.tile([128, SLAB], F16, tag="vts", bufs=3)
                if with_bv:
                    nc.scalar.activation(vts[:], vtp[:], AF.Identity,
                                         bias=bv_s[:], scale=1.0)
                else:
                    nc.scalar.activation(vts[:], vtp[:], AF.Copy,
                                         bias=0.0, scale=1.0)
                return vts

            def tail_b(g, vts, sc):
                t0 = g * SLAB
                last = g == nslab - 1
                # step 4: Y = V @ M.T; on the fast path the RMS scale is
                # applied here (tokens are partitions in py/ysb).  ps_y has
                # 4 bufs so all four matmuls issue back-to-back and the
                # copies trail on vector/scalar without stalling the PE.
                ysb = work.tile([128, OUT], F16, tag="ysb", bufs=6)
                pys = []
                for n in range(N_OUT_T):
                    py = ps_y.tile([128, 512], F32, tag="py",
                                   name=f"py{g}_{n}")
                    nc.tensor.matmul(py[:], vts[:],
                                     mt_s[:, n * 512:(n + 1) * 512],
                                     start=True, stop=True)
                    pys.append(py)
                for n in range(N_OUT_T):
                    py = pys[n]
                    ysl = ysb[:, n * 512:(n + 1) * 512]
                    if with_bv:
                        if n % 2 == 0:
                            nc.vector.tensor_copy(ysl, py[:])
                        else:
                            nc.scalar.activation(ysl, py[:], AF.Copy,
                                                 bias=0.0, scale=1.0)
                    else:
                        if n == 1:
                            nc.scalar.activation(ysl, py[:], AF.Copy,
                                                 bias=0.0, scale=sc[:])
                        else:
                            nc.vector.tensor_scalar_mul(ysl, py[:], sc[:])
                    if last and n == 1:
                        # final slab: overlap the first output half with the
                        # remaining copies so only 0.25 MB trails
                        nc.sync.dma_start(y_d[t0:t0 + SLAB, 0:1024],
                                          ysb[:, 0:1024])
                if last:
                    nc.sync.dma_start(y_d[t0:t0 + SLAB, 1024:2048],
                                      ysb[:, 1024:2048])
                else:
                    nc.sync.dma_start(y_d[t0:t0 + SLAB, :], ysb[:])

            # software pipeline: per iteration the PE sees
            #   [step1(g), transposes/step2(g-1), step4(g-1)]
            # with slab g's stats emitted between the two tail halves so the
            # vector/scalar queues stay in ready-order and only one tail
            # remains after the last step1.
            prev = None   # (g, ttn, sc, vts) carried between iterations
            sts = {}
            for g in range(nslab):
                pt = step1(g, junk_per_chunk=(SLAB0_JUNK if g == 0 else 0))
                if g >= 1:
                    pg, pttn, psc = sts.pop(g - 1)
                    vts = tail_a(pg, pttn)
                    sts[g] = (g,) + stat(g, pt)
                    tail_b(pg, vts, psc)
                else:
                    sts[g] = (g,) + stat(g, pt)
            pg, pttn, psc = sts.pop(nslab - 1)
            vts = tail_a(pg, pttn)
            tail_b(pg, vts, psc)

    nc.compile()
    return nc


def _host_prep(inputs):
    """Fold weights, swizzle X into fp16 token slabs, shard across cores."""
    h = np.asarray(inputs["hidden_states"], dtype=np.float32)
    b, s, hid = h.shape
    assert hid == HID
    x = np.ascontiguousarray(h.reshape(b * s, hid))
    ntok = b * s
    tok = ntok // N_CORES
    nslab = tok // SLAB

    kv_a_w = np.asarray(inputs["kv_a_w"], np.float32)
    kv_a_b = np.asarray(inputs["kv_a_b"], np.float32)
    kv_norm_w = np.asarray(inputs["kv_norm_w"], np.float32)
    kv_b_w = np.asarray(inputs["kv_b_w"], np.float32)
    kv_b_b = np.asarray(inputs["kv_b_b"], np.float32)
    o_w = np.asarray(inputs["o_w"], np.float32)

    w1s = np.ascontiguousarray(
        kv_a_w.T.reshape(N_HID_CK, 128, KV).transpose(1, 0, 2)
    ).astype(np.float16)
    wv = kv_b_w[D:2 * D] * (1.0 + kv_norm_w)[None, :]
    wvt = np.ascontiguousarray(wv.T).astype(np.float16)
    M = o_w.reshape(HID, 16, D).sum(axis=1)
    mt = np.ascontiguousarray(M.T).astype(np.float16)
    bv = np.ascontiguousarray(kv_b_b[D:2 * D].reshape(D, 1)).astype(np.float32)
    with_ba = bool(np.any(kv_a_b != 0.0))
    with_bv = bool(np.any(kv_b_b[D:2 * D] != 0.0))
    ba_row = np.ascontiguousarray(kv_a_b.reshape(1, KV)).astype(np.float16)
    ones_row = np.ones((1, 128), np.float16)

    in_maps = []
    for i in range(N_CORES):
        shard = x[i * tok:(i + 1) * tok]
        # [pair, hid_row, sub_slab, hid_chunk, token]
        xts = np.ascontiguousarray(
            shard.T.reshape(N_HID_CK, 128, nslab // 2, 2, SLAB)
            .transpose(2, 1, 3, 0, 4)
        ).astype(np.float16)
        m = {"xts": xts, "w1s": w1s, "wvt": wvt, "mt": mt, "bv": bv}
        if with_ba:
            m["bar"] = ba_row
            m["onesr"] = ones_row
        in_maps.append(m)

    def gather(results):
        y = np.concatenate([r["y"] for r in results], axis=0)
        return np.ascontiguousarray(y.reshape(b, s, HID).astype(np.float32))

    return in_maps, gather, with_ba, with_bv, tok


def _run(inputs, trace=False, **spmd_kwargs):
    in_maps, gather, with_ba, with_bv, tok = _host_prep(inputs)
    key = (tok, with_ba, with_bv)
    if key not in _NC_CACHE:
        _NC_CACHE[key] = _build_nc(tok, with_ba, with_bv)
    nc = _NC_CACHE[key]
    res = run_bass_kernel_spmd(nc, in_maps, core_ids=list(range(N_CORES)),
                               trace=trace, **spmd_kwargs)
    return gather(res.results), res


def kernel(**inputs) -> np.ndarray:
    y, _ = _run(inputs, trace=False)
    return y
